# revision 9
# baseline (speedup 1.0000x reference)
"""BasicMutualTransformerBlock on 8 trn2 NeuronCores.

Sharding: head-parallel attention (core h owns head h of every attention),
token-parallel everything per-token (LN stats, out-proj, residual, GEGLU FF:
core c owns tokens [256c, 256c+256)). Cross-core exchange is done on the host
between launches (collective_compute does not compile on this toolchain).

5 launches, 4 distinct SPMD programs:
  P_prep : LN stats + centering for each core's token slice of x1/x2
  P_attn : q/k/v projections + SDPA for one head of (mutual, self) attention
           (used twice: m1+a1, then m2+a2)
  P_mid  : out-proj + residual + LN stats/centering for the next stage
  P_tail : out-proj + residual + LN + both GEGLU FFs + final outputs
LayerNorm is folded: gamma into the following weight matrices (host-side),
beta/biases are all zero in this model (asserted).
"""
import sys
import numpy as np
import ml_dtypes

sys.path.insert(0, '/opt/trn_rl_repo')

import concourse.bass as bass  # noqa: E402
import concourse.mybir as mybir  # noqa: E402
import concourse.tile as tile  # noqa: E402
from concourse import bacc  # noqa: E402
from concourse.bass_utils import run_bass_kernel_spmd  # noqa: E402

P = 128
NTOK = 2048
DIM = 512
H = 8
DH = 64
FFI = 2048          # GEGLU inner dim
EPS = 1e-5
SC = DH ** -0.5
NC = 8
TS = NTOK // NC     # 256-token slice per core
KC = DIM // P       # 4 feature chunks
F32 = mybir.dt.float32
F32R = mybir.dt.float32r
BF = mybir.dt.bfloat16
AF = mybir.ActivationFunctionType
ALU = mybir.AluOpType


def _mm(nc, out, lhsT, rhs, start=True, stop=True):
    nc.tensor.matmul(out, lhsT, rhs, start=start, stop=stop)


def _load_fmajor(nc, sb, dram_ap, width):
    """DRAM [DIM, width] -> SBUF tile [128, KC, width]."""
    nc.sync.dma_start(sb[:, :, :], dram_ap.rearrange("(c p) t -> p c t", p=P))


def _store_fmajor(nc, dram_ap, sb):
    nc.sync.dma_start(dram_ap.rearrange("(c p) t -> p c t", p=P), sb[:, :, :])


def _stats_center(nc, sp, pp, ones, src, dst, uid, eps=None):
    """dst = (src - mean) * rsqrt(var + eps), per token (free axis).
    src/dst: SBUF [128, KC, TS] feature-major."""
    sumx = pp.tile([1, TS], F32, tag='st', name=f'sumx{uid}')
    for kc in range(KC):
        _mm(nc, sumx[:], ones[:, 0:1], src[:, kc, :], start=(kc == 0),
            stop=(kc == KC - 1))
    sq = sp.tile([P, KC, TS], F32, tag='sq', name=f'sq{uid}')
    nc.vector.tensor_tensor(sq[:], src[:], src[:], ALU.mult)
    sumsq = pp.tile([1, TS], F32, tag='st', name=f'sumsq{uid}')
    for kc in range(KC):
        _mm(nc, sumsq[:], ones[:, 0:1], sq[:, kc, :], start=(kc == 0),
            stop=(kc == KC - 1))
    mu = sp.tile([1, TS], F32, tag='mu', name=f'mu{uid}')
    nc.scalar.activation(mu[:], sumx[:], AF.Copy, scale=1.0 / DIM)
    ex2 = sp.tile([1, TS], F32, tag='ex2', name=f'ex2{uid}')
    nc.scalar.activation(ex2[:], sumsq[:], AF.Copy, scale=1.0 / DIM)
    musq = sp.tile([1, TS], F32, tag='musq', name=f'musq{uid}')
    nc.vector.tensor_tensor(musq[:], mu[:], mu[:], ALU.mult)
    var = sp.tile([1, TS], F32, tag='var', name=f'var{uid}')
    nc.vector.tensor_tensor(var[:], ex2[:], musq[:], ALU.subtract)
    sd = sp.tile([1, TS], F32, tag='sd', name=f'sd{uid}')
    nc.scalar.activation(sd[:], var[:], AF.Sqrt, bias=eps[0:1, 0:1])
    rs = sp.tile([1, TS], F32, tag='rs', name=f'rs{uid}')
    nc.vector.reciprocal(rs[:], sd[:])
    # broadcast mu/rs over 128 partitions via PE outer product with ones
    bmu = pp.tile([P, TS], F32, tag='bc', name=f'bmu{uid}')
    _mm(nc, bmu[:], ones[0:1, :], mu[:])
    brs = pp.tile([P, TS], F32, tag='bc', name=f'brs{uid}')
    _mm(nc, brs[:], ones[0:1, :], rs[:])
    for kc in range(KC):
        nc.vector.tensor_tensor(dst[:, kc, :], src[:, kc, :], bmu[:],
                                ALU.subtract)
        nc.vector.tensor_tensor(dst[:, kc, :], dst[:, kc, :], brs[:],
                                ALU.mult)


def _build_prep():
    nc = bacc.Bacc("TRN2", target_bir_lowering=False, debug=False,
                   num_devices=NC)
    din = {n: nc.dram_tensor(n, [DIM, TS], F32, kind="ExternalInput").ap()
           for n in ('x1s', 'x2s')}
    din['ones'] = nc.dram_tensor('ones', [P, P], F32, kind="ExternalInput").ap()
    din['epsv'] = nc.dram_tensor('epsv', [1, 1], F32, kind="ExternalInput").ap()
    dout = {n: nc.dram_tensor(n, [DIM, TS], BF, kind="ExternalOutput").ap()
            for n in ('cx1s', 'cx2s')}
    with tile.TileContext(nc) as tc:
        with tc.tile_pool(name='sp', bufs=2) as sp, \
             tc.tile_pool(name='pp', bufs=2, space='PSUM') as pp:
            ones = sp.tile([P, P], F32, tag='ones', name='ones')
            nc.sync.dma_start(ones[:], din['ones'][:])
            eps = sp.tile([1, 1], F32, tag='eps', name='eps')
            nc.sync.dma_start(eps[:], din['epsv'][:])
            for i, (xin, cout) in enumerate((('x1s', 'cx1s'),
                                             ('x2s', 'cx2s'))):
                xs = sp.tile([P, KC, TS], F32, tag='xs', name=f'xs{i}')
                _load_fmajor(nc, xs, din[xin], TS)
                cs = sp.tile([P, KC, TS], BF, tag='cs', name=f'cs{i}')
                _stats_center(nc, sp, pp, ones, xs, cs, uid=i, eps=eps)
                _store_fmajor(nc, dout[cout], cs)
    nc.compile()
    return nc


def _build_attn():
    """One head of mutual attn (kv from cx1+cx2) and one head of self attn
    (kv from cx2). Weights arrive pre-sliced per head; q weights pre-scaled."""
    nc = bacc.Bacc("TRN2", target_bir_lowering=False, debug=False,
                   num_devices=NC)
    ins = {}
    for n in ('cx1', 'cx2'):
        ins[n] = nc.dram_tensor(n, [DIM, NTOK], BF, kind="ExternalInput").ap()
    ins['ones'] = nc.dram_tensor('ones', [P, P], F32, kind="ExternalInput").ap()
    for n in ('m_wq', 'm_wk1', 'm_wv1', 'm_wk2', 'm_wv2',
              'a_wq', 'a_wk', 'a_wv'):
        ins[n] = nc.dram_tensor(n, [DIM, DH], BF, kind="ExternalInput").ap()
    outs = {n: nc.dram_tensor(n, [DH, NTOK], BF, kind="ExternalOutput").ap()
            for n in ('m_attn', 'a_attn')}
    with tile.TileContext(nc) as tc:
        with tc.tile_pool(name='sp', bufs=2) as sp, \
             tc.tile_pool(name='xp', bufs=1) as xp, \
             tc.tile_pool(name='wp', bufs=2) as wp, \
             tc.tile_pool(name='ep', bufs=3) as ep, \
             tc.tile_pool(name='pp', bufs=2, space='PSUM') as pp, \
             tc.tile_pool(name='psp', bufs=2, space='PSUM') as psp, \
             tc.tile_pool(name='pvp', bufs=1, space='PSUM') as pvp:
            ones = sp.tile([P, P], F32, tag='ones', name='ones')
            nc.sync.dma_start(ones[:], ins['ones'][:])
            onesb = sp.tile([P, P], BF, tag='onesb', name='onesb')
            nc.scalar.activation(onesb[:], ones[:], AF.Copy)
            xh1 = xp.tile([P, KC, NTOK], BF, tag='xh1', name='xh1')
            _load_fmajor(nc, xh1, ins['cx1'], NTOK)
            xh2 = xp.tile([P, KC, NTOK], BF, tag='xh2', name='xh2')
            _load_fmajor(nc, xh2, ins['cx2'], NTOK)

            stages = [
                ('m', xh1, [(xh1, 'm_wk1', 'm_wv1'), (xh2, 'm_wk2', 'm_wv2')],
                 'm_wq', outs['m_attn']),
                ('a', xh2, [(xh2, 'a_wk', 'a_wv')], 'a_wq', outs['a_attn']),
            ]
            for pref, qsrc, kvsrcs, wqn, attn_out in stages:
                nkv = NTOK * len(kvsrcs)
                nkc = nkv // P
                wq = wp.tile([P, KC, DH], BF, tag='w', name=f'{pref}wq')
                nc.sync.dma_start(
                    wq[:], ins[wqn].rearrange("(c p) m -> p c m", p=P))
                # q_T [64, NTOK]
                q = xp.tile([DH, NTOK], BF, tag='q', name=f'{pref}q')
                for nb in range(NTOK // 512):
                    pq = pp.tile([DH, 512], F32, tag='pj', name=f'{pref}pq{nb}')
                    for kc in range(KC):
                        _mm(nc, pq[:], wq[:, kc, :],
                            qsrc[:, kc, 512 * nb:512 * nb + 512],
                            start=(kc == 0), stop=(kc == KC - 1))
                    nc.scalar.activation(q[:, 512 * nb:512 * nb + 512], pq[:],
                                         AF.Copy)
                # k_T [64, nkv] and v [keys, 65] (v chunked [128, nkc, 65])
                kT = xp.tile([DH, 4096], BF, tag='kT', name=f'{pref}kT')
                v = xp.tile([P, 32, DH + 1], BF, tag='v', name=f'{pref}v')
                nc.scalar.activation(
                    v[:, 0:nkc, DH:DH + 1].rearrange("p c x -> p (c x)"),
                    onesb[:, 0:nkc], AF.Copy)
                for si, (xsrc, wkn, wvn) in enumerate(kvsrcs):
                    wk = wp.tile([P, KC, DH], BF, tag='w', name=f'{pref}wk{si}')
                    nc.sync.dma_start(
                        wk[:], ins[wkn].rearrange("(c p) m -> p c m", p=P))
                    wv = wp.tile([P, KC, DH], BF, tag='w', name=f'{pref}wv{si}')
                    nc.sync.dma_start(
                        wv[:], ins[wvn].rearrange("(c p) m -> p c m", p=P))
                    for nb in range(NTOK // 512):
                        pk = pp.tile([DH, 512], F32, tag='pj',
                                     name=f'{pref}pk{si}_{nb}')
                        for kc in range(KC):
                            _mm(nc, pk[:], wk[:, kc, :],
                                xsrc[:, kc, 512 * nb:512 * nb + 512],
                                start=(kc == 0), stop=(kc == KC - 1))
                        nc.scalar.activation(
                            kT[:, 2048 * si + 512 * nb:
                               2048 * si + 512 * nb + 512], pk[:], AF.Copy)
                    for tch in range(NTOK // P):
                        pv_ = pp.tile([P, DH], F32, tag='pj',
                                      name=f'{pref}pv{si}_{tch}')
                        for kc in range(KC):
                            _mm(nc, pv_[:], xsrc[:, kc, P * tch:P * tch + P],
                                wv[:, kc, :],
                                start=(kc == 0), stop=(kc == KC - 1))
                        nc.scalar.activation(v[:, 16 * si + tch, 0:DH],
                                             pv_[:], AF.Copy)
                # SDPA with denominator via the appended ones-column of v
                attn_sb = xp.tile([DH, NTOK], BF, tag='attn',
                                  name=f'{pref}attn')
                for qh in range(2):
                    q0 = 1024 * qh
                    acc = pvp.tile([DH + 1, 1024], F32, tag='pv',
                                   name=f'{pref}acc{qh}')
                    for kc in range(nkc):
                        scp = psp.tile([P, 1024], F32, tag='sc',
                                       name=f'{pref}sc{qh}_{kc}')
                        for nb in range(2):
                            _mm(nc, scp[:, 512 * nb:512 * nb + 512],
                                kT[:, P * kc:P * kc + P],
                                q[:, q0 + 512 * nb:q0 + 512 * nb + 512])
                        ex = ep.tile([P, 1024], BF, tag='ex',
                                     name=f'{pref}ex{qh}_{kc}')
                        nc.scalar.activation(ex[:], scp[:], AF.Exp)
                        for nb in range(2):
                            _mm(nc, acc[:, 512 * nb:512 * nb + 512],
                                v[:, kc, :], ex[:, 512 * nb:512 * nb + 512],
                                start=(kc == 0), stop=(kc == nkc - 1))
                    rec = sp.tile([1, 1024], F32, tag='rec',
                                  name=f'{pref}rec{qh}')
                    nc.vector.reciprocal(rec[:], acc[DH:DH + 1, :])
                    bc = psp.tile([P, 1024], F32, tag='sc',
                                  name=f'{pref}bc{qh}')
                    for nb in range(2):
                        _mm(nc, bc[0:DH, 512 * nb:512 * nb + 512],
                            ones[0:1, 0:DH],
                            rec[:, 512 * nb:512 * nb + 512])
                    num = sp.tile([DH, 1024], F32, tag='num',
                                  name=f'{pref}num{qh}')
                    nc.scalar.activation(num[:], acc[0:DH, :], AF.Copy)
                    nc.vector.tensor_tensor(attn_sb[:, q0:q0 + 1024],
                                            num[:], bc[0:DH, :],
                                            ALU.mult)
                nc.sync.dma_start(attn_out[:, :], attn_sb[:, :])
    nc.compile()
    return nc


def _outproj_resid(nc, sp, wp, pp, asl, wo_ap, r, uid):
    """r += W_o.T @ attn_slice ; asl [128, KC, TS], r [128, KC, TS]."""
    wo = wp.tile([P, KC, DIM], BF, tag='wo', name=f'wo{uid}')
    nc.sync.dma_start(wo[:], wo_ap.rearrange("(c p) m -> p c m", p=P))
    for mo in range(KC):
        po = pp.tile([P, TS], F32, tag='op', name=f'po{uid}_{mo}')
        for kc in range(KC):
            _mm(nc, po[:], wo[:, kc, P * mo:P * mo + P], asl[:, kc, :],
                start=(kc == 0), stop=(kc == KC - 1))
        nc.vector.tensor_tensor(r[:, mo, :], r[:, mo, :], po[:], ALU.add)


def _build_mid():
    nc = bacc.Bacc("TRN2", target_bir_lowering=False, debug=False,
                   num_devices=NC)
    ins = {}
    for n in ('x1s', 'x2s'):
        ins[n] = nc.dram_tensor(n, [DIM, TS], F32, kind="ExternalInput").ap()
    for n in ('m_asl', 'a_asl'):
        ins[n] = nc.dram_tensor(n, [DIM, TS], BF, kind="ExternalInput").ap()
    ins['ones'] = nc.dram_tensor('ones', [P, P], F32, kind="ExternalInput").ap()
    ins['epsv'] = nc.dram_tensor('epsv', [1, 1], F32, kind="ExternalInput").ap()
    for n in ('wo1', 'wo2'):
        ins[n] = nc.dram_tensor(n, [DIM, DIM], BF, kind="ExternalInput").ap()
    outs = {}
    for n in ('x1n', 'x2n'):
        outs[n] = nc.dram_tensor(n, [DIM, TS], F32, kind="ExternalOutput").ap()
    for n in ('cx1s', 'cx2s'):
        outs[n] = nc.dram_tensor(n, [DIM, TS], BF, kind="ExternalOutput").ap()
    with tile.TileContext(nc) as tc:
        with tc.tile_pool(name='sp', bufs=2) as sp, \
             tc.tile_pool(name='wp', bufs=2) as wp, \
             tc.tile_pool(name='pp', bufs=2, space='PSUM') as pp:
            ones = sp.tile([P, P], F32, tag='ones', name='ones')
            nc.sync.dma_start(ones[:], ins['ones'][:])
            eps = sp.tile([1, 1], F32, tag='eps', name='eps')
            nc.sync.dma_start(eps[:], ins['epsv'][:])
            for i, (an, xn, won, xon, con) in enumerate(
                    (('m_asl', 'x1s', 'wo1', 'x1n', 'cx1s'),
                     ('a_asl', 'x2s', 'wo2', 'x2n', 'cx2s'))):
                asl = sp.tile([P, KC, TS], BF, tag='asl', name=f'asl{i}')
                _load_fmajor(nc, asl, ins[an], TS)
                r = sp.tile([P, KC, TS], F32, tag='r', name=f'r{i}')
                _load_fmajor(nc, r, ins[xn], TS)
                _outproj_resid(nc, sp, wp, pp, asl, ins[won], r, uid=i)
                _store_fmajor(nc, outs[xon], r)
                cs = sp.tile([P, KC, TS], BF, tag='cs', name=f'cs{i}')
                _stats_center(nc, sp, pp, ones, r, cs, uid=i, eps=eps)
                _store_fmajor(nc, outs[con], cs)
    nc.compile()
    return nc


def _build_tail():
    nc = bacc.Bacc("TRN2", target_bir_lowering=False, debug=False,
                   num_devices=NC)
    ins = {}
    for n in ('x1s', 'x2s'):
        ins[n] = nc.dram_tensor(n, [DIM, TS], F32, kind="ExternalInput").ap()
    for n in ('m_asl', 'a_asl'):
        ins[n] = nc.dram_tensor(n, [DIM, TS], BF, kind="ExternalInput").ap()
    ins['ones'] = nc.dram_tensor('ones', [P, P], F32, kind="ExternalInput").ap()
    ins['epsv'] = nc.dram_tensor('epsv', [1, 1], F32, kind="ExternalInput").ap()
    for n in ('wo1', 'wo2'):
        ins[n] = nc.dram_tensor(n, [DIM, DIM], BF, kind="ExternalInput").ap()
    for n in ('w1m', 'w1f'):
        ins[n] = nc.dram_tensor(n, [DIM, 2 * FFI], BF,
                                kind="ExternalInput").ap()
    for n in ('w2m', 'w2f'):
        ins[n] = nc.dram_tensor(n, [FFI, DIM], BF, kind="ExternalInput").ap()
    outs = {n: nc.dram_tensor(n, [DIM, TS], F32, kind="ExternalOutput").ap()
            for n in ('y1s', 'y2s')}
    with tile.TileContext(nc) as tc:
        with tc.tile_pool(name='sp', bufs=2) as sp, \
             tc.tile_pool(name='wp', bufs=3) as wp, \
             tc.tile_pool(name='pp', bufs=2, space='PSUM') as pp:
            ones = sp.tile([P, P], F32, tag='ones', name='ones')
            nc.sync.dma_start(ones[:], ins['ones'][:])
            eps = sp.tile([1, 1], F32, tag='eps', name='eps')
            nc.sync.dma_start(eps[:], ins['epsv'][:])
            for i, (an, xn, won, w1n, w2n, yn) in enumerate(
                    (('m_asl', 'x1s', 'wo1', 'w1m', 'w2m', 'y1s'),
                     ('a_asl', 'x2s', 'wo2', 'w1f', 'w2f', 'y2s'))):
                asl = sp.tile([P, KC, TS], BF, tag='asl', name=f'asl{i}')
                _load_fmajor(nc, asl, ins[an], TS)
                r = sp.tile([P, KC, TS], F32, tag='r', name=f'r{i}')
                _load_fmajor(nc, r, ins[xn], TS)
                _outproj_resid(nc, sp, wp, pp, asl, ins[won], r, uid=i)
                cs = sp.tile([P, KC, TS], BF, tag='cs', name=f'cs{i}')
                _stats_center(nc, sp, pp, ones, r, cs, uid=i, eps=eps)
                # GEGLU FF: inner = (cs@W1a) * gelu(cs@W1g); y = r + inner@W2
                w1 = ins[w1n]
                inner = sp.tile([P, FFI // P, TS], BF, tag='inner',
                                name=f'inner{i}')
                for fi in range(FFI // P):
                    wa = wp.tile([P, KC, P], BF, tag='w1b', name=f'wa{i}_{fi}')
                    nc.sync.dma_start(
                        wa[:], w1[:, P * fi:P * fi + P]
                        .rearrange("(c p) m -> p c m", p=P))
                    wg = wp.tile([P, KC, P], BF, tag='w1b', name=f'wg{i}_{fi}')
                    nc.sync.dma_start(
                        wg[:], w1[:, FFI + P * fi:FFI + P * fi + P]
                        .rearrange("(c p) m -> p c m", p=P))
                    pa = pp.tile([P, TS], F32, tag='op', name=f'pa{i}_{fi}')
                    pg = pp.tile([P, TS], F32, tag='op', name=f'pg{i}_{fi}')
                    for kc in range(KC):
                        _mm(nc, pa[:], wa[:, kc, :], cs[:, kc, :],
                            start=(kc == 0), stop=(kc == KC - 1))
                    for kc in range(KC):
                        _mm(nc, pg[:], wg[:, kc, :], cs[:, kc, :],
                            start=(kc == 0), stop=(kc == KC - 1))
                    gl = sp.tile([P, TS], F32, tag='gl', name=f'gl{i}_{fi}')
                    nc.scalar.activation(gl[:], pg[:], AF.Gelu)
                    nc.vector.tensor_tensor(inner[:, fi, :], pa[:], gl[:],
                                            ALU.mult)
                w2 = ins[w2n]
                for mo in range(KC):
                    po = pp.tile([P, TS], F32, tag='op', name=f'py{i}_{mo}')
                    for fi in range(FFI // P):
                        w2b = wp.tile([P, P], BF, tag='w2b',
                                      name=f'w2b{i}_{mo}_{fi}')
                        nc.sync.dma_start(
                            w2b[:], w2[P * fi:P * fi + P, P * mo:P * mo + P])
                        _mm(nc, po[:], w2b[:], inner[:, fi, :],
                            start=(fi == 0), stop=(fi == FFI // P - 1))
                    yt = sp.tile([P, TS], F32, tag='yt', name=f'yt{i}_{mo}')
                    nc.vector.tensor_tensor(yt[:], r[:, mo, :], po[:], ALU.add)
                    nc.sync.dma_start(
                        outs[yn].rearrange("(c p) t -> p c t", p=P)[:, mo, :],
                        yt[:])
    nc.compile()
    return nc


_programs = {}


def _get_programs():
    if not _programs:
        _programs['prep'] = _build_prep()
        _programs['attn'] = _build_attn()
        _programs['mid'] = _build_mid()
        _programs['tail'] = _build_tail()
    return _programs


_ONES = np.ones((P, P), np.float32)
_EPSV = np.full((1, 1), EPS, np.float32)


def _run(prog, in_maps, extra=('ones', 'epsv')):
    for m in in_maps:
        m['ones'] = _ONES
        if 'epsv' in extra:
            m['epsv'] = _EPSV
    return run_bass_kernel_spmd(prog, in_maps,
                                core_ids=list(range(NC))).results


BFP = ml_dtypes.bfloat16


def _f(a):
    return np.ascontiguousarray(np.asarray(a, dtype=np.float32))


def _bf(a):
    return np.ascontiguousarray(np.asarray(a).astype(BFP))


def kernel(x1, x2, params):
    progs = _get_programs()
    p = {k: {kk: _f(vv) for kk, vv in v.items()} for k, v in params.items()}
    # all LN betas / projection biases are zero in this model; gamma folds
    for n in ('n1', 'n2', 'n3'):
        assert np.abs(p[n]['b']).max() == 0.0
    for n in ('m1', 'm2', 'a1', 'a2'):
        assert np.abs(p[n]['ob']).max() == 0.0
    for n in ('mff', 'ff'):
        assert np.abs(p[n]['b1']).max() == 0.0 and np.abs(p[n]['b2']).max() == 0.0

    g1 = p['n1']['g'][:, None]
    g2 = p['n2']['g'][:, None]
    g3 = p['n3']['g'][:, None]

    x1f = _f(x1)[0].T  # [DIM, NTOK]
    x2f = _f(x2)[0].T

    def tok_slices(a):
        return [np.ascontiguousarray(a[:, TS * c:TS * c + TS])
                for c in range(NC)]

    def hsl(w, c):
        return np.ascontiguousarray(w[:, DH * c:DH * c + DH])

    x1sl, x2sl = tok_slices(x1f), tok_slices(x2f)

    # L1: per-slice LN stats/centering of the raw inputs (norm n1)
    r1 = _run(progs['prep'],
              [{'x1s': x1sl[c], 'x2s': x2sl[c]} for c in range(NC)])
    cx1 = np.concatenate([r1[c]['cx1s'] for c in range(NC)], axis=1)
    cx2 = np.concatenate([r1[c]['cx2s'] for c in range(NC)], axis=1)

    def attn_launch(cxa, cxb, mw, aw):
        # mutual head weights from module mw (q from stream a; k1/v1 stream a,
        # k2/v2 stream b), self head weights from module aw (all stream b)
        maps = []
        for c in range(NC):
            maps.append({
                'cx1': cxa, 'cx2': cxb,
                'm_wq': _bf(hsl(mw['wq'], c)), 'm_wk1': _bf(hsl(mw['wk1'], c)),
                'm_wv1': _bf(hsl(mw['wv1'], c)), 'm_wk2': _bf(hsl(mw['wk2'], c)),
                'm_wv2': _bf(hsl(mw['wv2'], c)),
                'a_wq': _bf(hsl(aw['wq'], c)), 'a_wk': _bf(hsl(aw['wk'], c)),
                'a_wv': _bf(hsl(aw['wv'], c)),
            })
        r = _run(progs['attn'], maps, extra=('ones',))
        m_attn = np.concatenate([r[c]['m_attn'] for c in range(NC)], axis=0)
        a_attn = np.concatenate([r[c]['a_attn'] for c in range(NC)], axis=0)
        return m_attn, a_attn

    def fold(mod, g):
        return {'wq': g * mod['q1' if 'q1' in mod else 'q'] * SC,
                'wk1': g * mod.get('k1', mod.get('k')),
                'wv1': g * mod.get('v1', mod.get('v')),
                'wk2': g * mod.get('k2', mod.get('k')),
                'wv2': g * mod.get('v2', mod.get('v'))}

    m1w = fold(p['m1'], g1)
    a1w = {'wq': g1 * p['a1']['q'] * SC, 'wk': g1 * p['a1']['k'],
           'wv': g1 * p['a1']['v']}
    m_attn1, a_attn1 = attn_launch(cx1, cx2, m1w, a1w)

    # L3: out-proj + residual + LN(n2)
    masl, aasl = tok_slices(m_attn1), tok_slices(a_attn1)
    r3 = _run(progs['mid'],
              [{'m_asl': masl[c], 'a_asl': aasl[c],
                'x1s': x1sl[c], 'x2s': x2sl[c],
                'wo1': _bf(p['m1']['ow']), 'wo2': _bf(p['a1']['ow'])}
               for c in range(NC)])
    x1psl = [r3[c]['x1n'] for c in range(NC)]
    x2psl = [r3[c]['x2n'] for c in range(NC)]
    cx1p = np.concatenate([r3[c]['cx1s'] for c in range(NC)], axis=1)
    cx2p = np.concatenate([r3[c]['cx2s'] for c in range(NC)], axis=1)

    m2w = fold(p['m2'], g2)
    a2w = {'wq': g2 * p['a2']['q'] * SC, 'wk': g2 * p['a2']['k'],
           'wv': g2 * p['a2']['v']}
    m_attn2, a_attn2 = attn_launch(cx1p, cx2p, m2w, a2w)

    # L5: out-proj + residual + LN(n3) + GEGLU FFs
    masl2, aasl2 = tok_slices(m_attn2), tok_slices(a_attn2)
    r5 = _run(progs['tail'],
              [{'m_asl': masl2[c], 'a_asl': aasl2[c],
                'x1s': x1psl[c], 'x2s': x2psl[c],
                'wo1': _bf(p['m2']['ow']), 'wo2': _bf(p['a2']['ow']),
                'w1m': _bf(g3 * p['mff']['w1']), 'w2m': _bf(p['mff']['w2']),
                'w1f': _bf(g3 * p['ff']['w1']), 'w2f': _bf(p['ff']['w2'])}
               for c in range(NC)])
    y1 = np.concatenate([r5[c]['y1s'].T for c in range(NC)], axis=0)[None]
    y2 = np.concatenate([r5[c]['y2s'].T for c in range(NC)], axis=0)[None]
    return (y1, y2)


# revision 10
# speedup vs baseline: 1.0054x; 1.0054x over previous
"""BasicMutualTransformerBlock on 8 trn2 NeuronCores.

Sharding: head-parallel attention (core h owns head h of every attention),
token-parallel everything per-token (LN stats, out-proj, residual, GEGLU FF:
core c owns tokens [256c, 256c+256)). Cross-core exchange is done on the host
between launches (collective_compute does not compile on this toolchain).

5 launches, 4 distinct SPMD programs:
  P_prep : LN stats + centering for each core's token slice of x1/x2
  P_attn : q/k/v projections + SDPA for one head of (mutual, self) attention
           (used twice: m1+a1, then m2+a2)
  P_mid  : out-proj + residual + LN stats/centering for the next stage
  P_tail : out-proj + residual + LN + both GEGLU FFs + final outputs
LayerNorm is folded: gamma into the following weight matrices (host-side),
beta/biases are all zero in this model (asserted).
"""
import sys
import numpy as np
import ml_dtypes

sys.path.insert(0, '/opt/trn_rl_repo')

import concourse.bass as bass  # noqa: E402
import concourse.mybir as mybir  # noqa: E402
import concourse.tile as tile  # noqa: E402
from concourse import bacc  # noqa: E402
from concourse.bass_utils import run_bass_kernel_spmd  # noqa: E402

P = 128
NTOK = 2048
DIM = 512
H = 8
DH = 64
FFI = 2048          # GEGLU inner dim
EPS = 1e-5
SC = DH ** -0.5
NC = 8
TS = NTOK // NC     # 256-token slice per core
KC = DIM // P       # 4 feature chunks
F32 = mybir.dt.float32
F32R = mybir.dt.float32r
BF = mybir.dt.bfloat16
AF = mybir.ActivationFunctionType
ALU = mybir.AluOpType


def _mm(nc, out, lhsT, rhs, start=True, stop=True):
    nc.tensor.matmul(out, lhsT, rhs, start=start, stop=stop)


def _load_fmajor(nc, sb, dram_ap, width):
    """DRAM [DIM, width] -> SBUF tile [128, KC, width]."""
    nc.sync.dma_start(sb[:, :, :], dram_ap.rearrange("(c p) t -> p c t", p=P))


def _store_fmajor(nc, dram_ap, sb):
    nc.sync.dma_start(dram_ap.rearrange("(c p) t -> p c t", p=P), sb[:, :, :])


def _stats_center(nc, sp, pp, ones, src, dst, uid, eps=None):
    """dst = (src - mean) * rsqrt(var + eps), per token (free axis).
    src/dst: SBUF [128, KC, TS] feature-major."""
    sumx = pp.tile([1, TS], F32, tag='st', name=f'sumx{uid}')
    for kc in range(KC):
        _mm(nc, sumx[:], ones[:, 0:1], src[:, kc, :], start=(kc == 0),
            stop=(kc == KC - 1))
    sq = sp.tile([P, KC, TS], F32, tag='sq', name=f'sq{uid}')
    nc.vector.tensor_tensor(sq[:], src[:], src[:], ALU.mult)
    sumsq = pp.tile([1, TS], F32, tag='st', name=f'sumsq{uid}')
    for kc in range(KC):
        _mm(nc, sumsq[:], ones[:, 0:1], sq[:, kc, :], start=(kc == 0),
            stop=(kc == KC - 1))
    mu = sp.tile([1, TS], F32, tag='mu', name=f'mu{uid}')
    nc.scalar.activation(mu[:], sumx[:], AF.Copy, scale=1.0 / DIM)
    ex2 = sp.tile([1, TS], F32, tag='ex2', name=f'ex2{uid}')
    nc.scalar.activation(ex2[:], sumsq[:], AF.Copy, scale=1.0 / DIM)
    musq = sp.tile([1, TS], F32, tag='musq', name=f'musq{uid}')
    nc.vector.tensor_tensor(musq[:], mu[:], mu[:], ALU.mult)
    var = sp.tile([1, TS], F32, tag='var', name=f'var{uid}')
    nc.vector.tensor_tensor(var[:], ex2[:], musq[:], ALU.subtract)
    sd = sp.tile([1, TS], F32, tag='sd', name=f'sd{uid}')
    nc.scalar.activation(sd[:], var[:], AF.Sqrt, bias=eps[0:1, 0:1])
    rs = sp.tile([1, TS], F32, tag='rs', name=f'rs{uid}')
    nc.vector.reciprocal(rs[:], sd[:])
    # broadcast mu/rs over 128 partitions via PE outer product with ones
    bmu = pp.tile([P, TS], F32, tag='bc', name=f'bmu{uid}')
    _mm(nc, bmu[:], ones[0:1, :], mu[:])
    brs = pp.tile([P, TS], F32, tag='bc', name=f'brs{uid}')
    _mm(nc, brs[:], ones[0:1, :], rs[:])
    for kc in range(KC):
        nc.vector.tensor_tensor(dst[:, kc, :], src[:, kc, :], bmu[:],
                                ALU.subtract)
        nc.vector.tensor_tensor(dst[:, kc, :], dst[:, kc, :], brs[:],
                                ALU.mult)


def _build_prep():
    nc = bacc.Bacc("TRN2", target_bir_lowering=False, debug=False,
                   num_devices=NC)
    din = {n: nc.dram_tensor(n, [DIM, TS], F32, kind="ExternalInput").ap()
           for n in ('x1s', 'x2s')}
    din['ones'] = nc.dram_tensor('ones', [P, P], F32, kind="ExternalInput").ap()
    din['epsv'] = nc.dram_tensor('epsv', [1, 1], F32, kind="ExternalInput").ap()
    dout = {n: nc.dram_tensor(n, [DIM, TS], BF, kind="ExternalOutput").ap()
            for n in ('cx1s', 'cx2s')}
    with tile.TileContext(nc) as tc:
        with tc.tile_pool(name='sp', bufs=2) as sp, \
             tc.tile_pool(name='pp', bufs=2, space='PSUM') as pp:
            ones = sp.tile([P, P], F32, tag='ones', name='ones')
            nc.sync.dma_start(ones[:], din['ones'][:])
            eps = sp.tile([1, 1], F32, tag='eps', name='eps')
            nc.sync.dma_start(eps[:], din['epsv'][:])
            for i, (xin, cout) in enumerate((('x1s', 'cx1s'),
                                             ('x2s', 'cx2s'))):
                xs = sp.tile([P, KC, TS], F32, tag='xs', name=f'xs{i}')
                _load_fmajor(nc, xs, din[xin], TS)
                cs = sp.tile([P, KC, TS], BF, tag='cs', name=f'cs{i}')
                _stats_center(nc, sp, pp, ones, xs, cs, uid=i, eps=eps)
                _store_fmajor(nc, dout[cout], cs)
    nc.compile()
    return nc


def _build_attn():
    """One head of mutual attn (kv from cx1+cx2) and one head of self attn
    (kv from cx2). Weights arrive pre-sliced per head; q weights pre-scaled."""
    nc = bacc.Bacc("TRN2", target_bir_lowering=False, debug=False,
                   num_devices=NC)
    ins = {}
    for n in ('cx1', 'cx2'):
        ins[n] = nc.dram_tensor(n, [DIM, NTOK], BF, kind="ExternalInput").ap()
    ins['ones'] = nc.dram_tensor('ones', [P, P], F32, kind="ExternalInput").ap()
    for n in ('m_wq', 'm_wk1', 'm_wv1', 'm_wk2', 'm_wv2',
              'a_wq', 'a_wk', 'a_wv'):
        ins[n] = nc.dram_tensor(n, [DIM, DH], BF, kind="ExternalInput").ap()
    outs = {n: nc.dram_tensor(n, [DH, NTOK], BF, kind="ExternalOutput").ap()
            for n in ('m_attn', 'a_attn')}
    with tile.TileContext(nc) as tc:
        with tc.tile_pool(name='sp', bufs=2) as sp, \
             tc.tile_pool(name='xp', bufs=1) as xp, \
             tc.tile_pool(name='wp', bufs=2) as wp, \
             tc.tile_pool(name='ep', bufs=3) as ep, \
             tc.tile_pool(name='pp', bufs=2, space='PSUM') as pp, \
             tc.tile_pool(name='psp', bufs=2, space='PSUM') as psp, \
             tc.tile_pool(name='pvp', bufs=1, space='PSUM') as pvp:
            ones = sp.tile([P, P], F32, tag='ones', name='ones')
            nc.sync.dma_start(ones[:], ins['ones'][:])
            onesb = sp.tile([P, P], BF, tag='onesb', name='onesb')
            nc.scalar.activation(onesb[:], ones[:], AF.Copy)
            xh1 = xp.tile([P, KC, NTOK], BF, tag='xh1', name='xh1')
            _load_fmajor(nc, xh1, ins['cx1'], NTOK)
            xh2 = xp.tile([P, KC, NTOK], BF, tag='xh2', name='xh2')
            _load_fmajor(nc, xh2, ins['cx2'], NTOK)

            stages = [
                ('m', xh1, [(xh1, 'm_wk1', 'm_wv1'), (xh2, 'm_wk2', 'm_wv2')],
                 'm_wq', outs['m_attn']),
                ('a', xh2, [(xh2, 'a_wk', 'a_wv')], 'a_wq', outs['a_attn']),
            ]
            for pref, qsrc, kvsrcs, wqn, attn_out in stages:
                nkv = NTOK * len(kvsrcs)
                nkc = nkv // P
                wq = wp.tile([P, KC, DH], BF, tag='w', name=f'{pref}wq')
                nc.sync.dma_start(
                    wq[:], ins[wqn].rearrange("(c p) m -> p c m", p=P))
                # q_T [64, NTOK]
                q = xp.tile([DH, NTOK], BF, tag='q', name=f'{pref}q')
                for nb in range(NTOK // 512):
                    pq = pp.tile([DH, 512], F32, tag='pj', name=f'{pref}pq{nb}')
                    for kc in range(KC):
                        _mm(nc, pq[:], wq[:, kc, :],
                            qsrc[:, kc, 512 * nb:512 * nb + 512],
                            start=(kc == 0), stop=(kc == KC - 1))
                    nc.scalar.activation(q[:, 512 * nb:512 * nb + 512], pq[:],
                                         AF.Copy)
                # k_T [64, nkv] and v [keys, 65] (v chunked [128, nkc, 65])
                kT = xp.tile([DH, 4096], BF, tag='kT', name=f'{pref}kT')
                v = xp.tile([P, 32, DH + 1], BF, tag='v', name=f'{pref}v')
                nc.scalar.activation(
                    v[:, 0:nkc, DH:DH + 1].rearrange("p c x -> p (c x)"),
                    onesb[:, 0:nkc], AF.Copy)
                for si, (xsrc, wkn, wvn) in enumerate(kvsrcs):
                    wk = wp.tile([P, KC, DH], BF, tag='w', name=f'{pref}wk{si}')
                    nc.sync.dma_start(
                        wk[:], ins[wkn].rearrange("(c p) m -> p c m", p=P))
                    wv = wp.tile([P, KC, DH], BF, tag='w', name=f'{pref}wv{si}')
                    nc.sync.dma_start(
                        wv[:], ins[wvn].rearrange("(c p) m -> p c m", p=P))
                    for nb in range(NTOK // 512):
                        pk = pp.tile([DH, 512], F32, tag='pj',
                                     name=f'{pref}pk{si}_{nb}')
                        for kc in range(KC):
                            _mm(nc, pk[:], wk[:, kc, :],
                                xsrc[:, kc, 512 * nb:512 * nb + 512],
                                start=(kc == 0), stop=(kc == KC - 1))
                        nc.scalar.activation(
                            kT[:, 2048 * si + 512 * nb:
                               2048 * si + 512 * nb + 512], pk[:], AF.Copy)
                    for tch in range(NTOK // P):
                        pv_ = pp.tile([P, DH], F32, tag='pj',
                                      name=f'{pref}pv{si}_{tch}')
                        for kc in range(KC):
                            _mm(nc, pv_[:], xsrc[:, kc, P * tch:P * tch + P],
                                wv[:, kc, :],
                                start=(kc == 0), stop=(kc == KC - 1))
                        nc.scalar.activation(v[:, 16 * si + tch, 0:DH],
                                             pv_[:], AF.Copy)
                # SDPA with denominator via the appended ones-column of v
                attn_sb = xp.tile([DH, NTOK], BF, tag='attn',
                                  name=f'{pref}attn')
                for qh in range(2):
                    q0 = 1024 * qh
                    acc = pvp.tile([DH + 1, 1024], F32, tag='pv',
                                   name=f'{pref}acc{qh}')
                    for kc in range(nkc):
                        scp = psp.tile([P, 1024], F32, tag='sc',
                                       name=f'{pref}sc{qh}_{kc}')
                        for nb in range(2):
                            _mm(nc, scp[:, 512 * nb:512 * nb + 512],
                                kT[:, P * kc:P * kc + P],
                                q[:, q0 + 512 * nb:q0 + 512 * nb + 512])
                        ex = ep.tile([P, 1024], BF, tag='ex',
                                     name=f'{pref}ex{qh}_{kc}')
                        nc.scalar.activation(ex[:], scp[:], AF.Exp)
                        for nb in range(2):
                            _mm(nc, acc[:, 512 * nb:512 * nb + 512],
                                v[:, kc, :], ex[:, 512 * nb:512 * nb + 512],
                                start=(kc == 0), stop=(kc == nkc - 1))
                    rec = sp.tile([1, 1024], F32, tag='rec',
                                  name=f'{pref}rec{qh}')
                    nc.vector.reciprocal(rec[:], acc[DH:DH + 1, :])
                    bc = psp.tile([P, 1024], F32, tag='sc',
                                  name=f'{pref}bc{qh}')
                    for nb in range(2):
                        _mm(nc, bc[0:DH, 512 * nb:512 * nb + 512],
                            ones[0:1, 0:DH],
                            rec[:, 512 * nb:512 * nb + 512])
                    num = sp.tile([DH, 1024], F32, tag='num',
                                  name=f'{pref}num{qh}')
                    nc.scalar.activation(num[:], acc[0:DH, :], AF.Copy)
                    nc.vector.tensor_tensor(attn_sb[:, q0:q0 + 1024],
                                            num[:], bc[0:DH, :],
                                            ALU.mult)
                nc.sync.dma_start(attn_out[:, :], attn_sb[:, :])
    nc.compile()
    return nc


def _outproj_resid(nc, sp, wp, pp, asl, wo_ap, r, uid):
    """r += W_o.T @ attn_slice ; asl [128, KC, TS], r [128, KC, TS]."""
    wo = wp.tile([P, KC, DIM], BF, tag='wo', name=f'wo{uid}')
    nc.sync.dma_start(wo[:], wo_ap.rearrange("(c p) m -> p c m", p=P))
    for mo in range(KC):
        po = pp.tile([P, TS], F32, tag='op', name=f'po{uid}_{mo}')
        for kc in range(KC):
            _mm(nc, po[:], wo[:, kc, P * mo:P * mo + P], asl[:, kc, :],
                start=(kc == 0), stop=(kc == KC - 1))
        nc.vector.tensor_tensor(r[:, mo, :], r[:, mo, :], po[:], ALU.add)


def _build_mid():
    nc = bacc.Bacc("TRN2", target_bir_lowering=False, debug=False,
                   num_devices=NC)
    ins = {}
    for n in ('x1s', 'x2s'):
        ins[n] = nc.dram_tensor(n, [DIM, TS], F32, kind="ExternalInput").ap()
    for n in ('m_asl', 'a_asl'):
        ins[n] = nc.dram_tensor(n, [DIM, TS], BF, kind="ExternalInput").ap()
    ins['ones'] = nc.dram_tensor('ones', [P, P], F32, kind="ExternalInput").ap()
    ins['epsv'] = nc.dram_tensor('epsv', [1, 1], F32, kind="ExternalInput").ap()
    for n in ('wo1', 'wo2'):
        ins[n] = nc.dram_tensor(n, [DIM, DIM], BF, kind="ExternalInput").ap()
    outs = {}
    for n in ('x1n', 'x2n'):
        outs[n] = nc.dram_tensor(n, [DIM, TS], F32, kind="ExternalOutput").ap()
    for n in ('cx1s', 'cx2s'):
        outs[n] = nc.dram_tensor(n, [DIM, TS], BF, kind="ExternalOutput").ap()
    with tile.TileContext(nc) as tc:
        with tc.tile_pool(name='sp', bufs=2) as sp, \
             tc.tile_pool(name='wp', bufs=2) as wp, \
             tc.tile_pool(name='pp', bufs=2, space='PSUM') as pp:
            ones = sp.tile([P, P], F32, tag='ones', name='ones')
            nc.sync.dma_start(ones[:], ins['ones'][:])
            eps = sp.tile([1, 1], F32, tag='eps', name='eps')
            nc.sync.dma_start(eps[:], ins['epsv'][:])
            for i, (an, xn, won, xon, con) in enumerate(
                    (('m_asl', 'x1s', 'wo1', 'x1n', 'cx1s'),
                     ('a_asl', 'x2s', 'wo2', 'x2n', 'cx2s'))):
                asl = sp.tile([P, KC, TS], BF, tag='asl', name=f'asl{i}')
                _load_fmajor(nc, asl, ins[an], TS)
                r = sp.tile([P, KC, TS], F32, tag='r', name=f'r{i}')
                _load_fmajor(nc, r, ins[xn], TS)
                _outproj_resid(nc, sp, wp, pp, asl, ins[won], r, uid=i)
                _store_fmajor(nc, outs[xon], r)
                cs = sp.tile([P, KC, TS], BF, tag='cs', name=f'cs{i}')
                _stats_center(nc, sp, pp, ones, r, cs, uid=i, eps=eps)
                _store_fmajor(nc, outs[con], cs)
    nc.compile()
    return nc


def _build_tail():
    nc = bacc.Bacc("TRN2", target_bir_lowering=False, debug=False,
                   num_devices=NC)
    ins = {}
    for n in ('x1s', 'x2s'):
        ins[n] = nc.dram_tensor(n, [DIM, TS], F32, kind="ExternalInput").ap()
    for n in ('m_asl', 'a_asl'):
        ins[n] = nc.dram_tensor(n, [DIM, TS], BF, kind="ExternalInput").ap()
    ins['ones'] = nc.dram_tensor('ones', [P, P], F32, kind="ExternalInput").ap()
    ins['epsv'] = nc.dram_tensor('epsv', [1, 1], F32, kind="ExternalInput").ap()
    for n in ('wo1', 'wo2'):
        ins[n] = nc.dram_tensor(n, [DIM, DIM], BF, kind="ExternalInput").ap()
    for n in ('w1m', 'w1f'):
        ins[n] = nc.dram_tensor(n, [DIM, 2 * FFI], BF,
                                kind="ExternalInput").ap()
    for n in ('w2m', 'w2f'):
        ins[n] = nc.dram_tensor(n, [FFI, DIM], BF, kind="ExternalInput").ap()
    outs = {n: nc.dram_tensor(n, [DIM, TS], F32, kind="ExternalOutput").ap()
            for n in ('y1s', 'y2s')}
    with tile.TileContext(nc) as tc:
        with tc.tile_pool(name='sp', bufs=2) as sp, \
             tc.tile_pool(name='wp', bufs=3) as wp, \
             tc.tile_pool(name='pp', bufs=2, space='PSUM') as pp:
            ones = sp.tile([P, P], F32, tag='ones', name='ones')
            nc.sync.dma_start(ones[:], ins['ones'][:])
            eps = sp.tile([1, 1], F32, tag='eps', name='eps')
            nc.sync.dma_start(eps[:], ins['epsv'][:])
            for i, (an, xn, won, w1n, w2n, yn) in enumerate(
                    (('m_asl', 'x1s', 'wo1', 'w1m', 'w2m', 'y1s'),
                     ('a_asl', 'x2s', 'wo2', 'w1f', 'w2f', 'y2s'))):
                asl = sp.tile([P, KC, TS], BF, tag='asl', name=f'asl{i}')
                _load_fmajor(nc, asl, ins[an], TS)
                r = sp.tile([P, KC, TS], F32, tag='r', name=f'r{i}')
                _load_fmajor(nc, r, ins[xn], TS)
                _outproj_resid(nc, sp, wp, pp, asl, ins[won], r, uid=i)
                cs = sp.tile([P, KC, TS], BF, tag='cs', name=f'cs{i}')
                _stats_center(nc, sp, pp, ones, r, cs, uid=i, eps=eps)
                # GEGLU FF: inner = (cs@W1a) * gelu(cs@W1g); y = r + inner@W2
                w1 = ins[w1n]
                inner = sp.tile([P, FFI // P, TS], BF, tag='inner',
                                name=f'inner{i}')
                for fi in range(FFI // P):
                    wa = wp.tile([P, KC, P], BF, tag='w1b', name=f'wa{i}_{fi}')
                    nc.sync.dma_start(
                        wa[:], w1[:, P * fi:P * fi + P]
                        .rearrange("(c p) m -> p c m", p=P))
                    wg = wp.tile([P, KC, P], BF, tag='w1b', name=f'wg{i}_{fi}')
                    nc.sync.dma_start(
                        wg[:], w1[:, FFI + P * fi:FFI + P * fi + P]
                        .rearrange("(c p) m -> p c m", p=P))
                    pa = pp.tile([P, TS], F32, tag='op', name=f'pa{i}_{fi}')
                    pg = pp.tile([P, TS], F32, tag='op', name=f'pg{i}_{fi}')
                    for kc in range(KC):
                        _mm(nc, pa[:], wa[:, kc, :], cs[:, kc, :],
                            start=(kc == 0), stop=(kc == KC - 1))
                    for kc in range(KC):
                        _mm(nc, pg[:], wg[:, kc, :], cs[:, kc, :],
                            start=(kc == 0), stop=(kc == KC - 1))
                    gl = sp.tile([P, TS], F32, tag='gl', name=f'gl{i}_{fi}')
                    nc.scalar.activation(gl[:], pg[:], AF.Gelu)
                    nc.vector.tensor_tensor(inner[:, fi, :], pa[:], gl[:],
                                            ALU.mult)
                w2 = ins[w2n]
                for mo in range(KC):
                    po = pp.tile([P, TS], F32, tag='op', name=f'py{i}_{mo}')
                    for fi in range(FFI // P):
                        w2b = wp.tile([P, P], BF, tag='w2b',
                                      name=f'w2b{i}_{mo}_{fi}')
                        nc.sync.dma_start(
                            w2b[:], w2[P * fi:P * fi + P, P * mo:P * mo + P])
                        _mm(nc, po[:], w2b[:], inner[:, fi, :],
                            start=(fi == 0), stop=(fi == FFI // P - 1))
                    yt = sp.tile([P, TS], F32, tag='yt', name=f'yt{i}_{mo}')
                    nc.vector.tensor_tensor(yt[:], r[:, mo, :], po[:], ALU.add)
                    nc.sync.dma_start(
                        outs[yn].rearrange("(c p) t -> p c t", p=P)[:, mo, :],
                        yt[:])
    nc.compile()
    return nc


_programs = {}


def _get_programs():
    if not _programs:
        _programs['prep'] = _build_prep()
        _programs['attn'] = _build_attn()
        _programs['mid'] = _build_mid()
        _programs['tail'] = _build_tail()
    return _programs


_ONES = np.ones((P, P), np.float32)
_EPSV = np.full((1, 1), EPS, np.float32)
TRACE = False
LAST_TIMES = []


def _run(prog, in_maps, extra=('ones', 'epsv')):
    for m in in_maps:
        m['ones'] = _ONES
        if 'epsv' in extra:
            m['epsv'] = _EPSV
    r = run_bass_kernel_spmd(prog, in_maps, core_ids=list(range(NC)),
                             trace=TRACE)
    if TRACE:
        LAST_TIMES.append(r.exec_time_ns)
    return r.results


BFP = ml_dtypes.bfloat16


def _f(a):
    return np.ascontiguousarray(np.asarray(a, dtype=np.float32))


def _bf(a):
    return np.ascontiguousarray(np.asarray(a).astype(BFP))


def kernel(x1, x2, params):
    progs = _get_programs()
    p = {k: {kk: _f(vv) for kk, vv in v.items()} for k, v in params.items()}
    # all LN betas / projection biases are zero in this model; gamma folds
    for n in ('n1', 'n2', 'n3'):
        assert np.abs(p[n]['b']).max() == 0.0
    for n in ('m1', 'm2', 'a1', 'a2'):
        assert np.abs(p[n]['ob']).max() == 0.0
    for n in ('mff', 'ff'):
        assert np.abs(p[n]['b1']).max() == 0.0 and np.abs(p[n]['b2']).max() == 0.0

    g1 = p['n1']['g'][:, None]
    g2 = p['n2']['g'][:, None]
    g3 = p['n3']['g'][:, None]

    x1f = _f(x1)[0].T  # [DIM, NTOK]
    x2f = _f(x2)[0].T

    def tok_slices(a):
        return [np.ascontiguousarray(a[:, TS * c:TS * c + TS])
                for c in range(NC)]

    def hsl(w, c):
        return np.ascontiguousarray(w[:, DH * c:DH * c + DH])

    x1sl, x2sl = tok_slices(x1f), tok_slices(x2f)

    # L1: per-slice LN stats/centering of the raw inputs (norm n1)
    r1 = _run(progs['prep'],
              [{'x1s': x1sl[c], 'x2s': x2sl[c]} for c in range(NC)])
    cx1 = np.concatenate([r1[c]['cx1s'] for c in range(NC)], axis=1)
    cx2 = np.concatenate([r1[c]['cx2s'] for c in range(NC)], axis=1)

    def attn_launch(cxa, cxb, mw, aw):
        # mutual head weights from module mw (q from stream a; k1/v1 stream a,
        # k2/v2 stream b), self head weights from module aw (all stream b)
        maps = []
        for c in range(NC):
            maps.append({
                'cx1': cxa, 'cx2': cxb,
                'm_wq': _bf(hsl(mw['wq'], c)), 'm_wk1': _bf(hsl(mw['wk1'], c)),
                'm_wv1': _bf(hsl(mw['wv1'], c)), 'm_wk2': _bf(hsl(mw['wk2'], c)),
                'm_wv2': _bf(hsl(mw['wv2'], c)),
                'a_wq': _bf(hsl(aw['wq'], c)), 'a_wk': _bf(hsl(aw['wk'], c)),
                'a_wv': _bf(hsl(aw['wv'], c)),
            })
        r = _run(progs['attn'], maps, extra=('ones',))
        m_attn = np.concatenate([r[c]['m_attn'] for c in range(NC)], axis=0)
        a_attn = np.concatenate([r[c]['a_attn'] for c in range(NC)], axis=0)
        return m_attn, a_attn

    def fold(mod, g):
        return {'wq': g * mod['q1' if 'q1' in mod else 'q'] * SC,
                'wk1': g * mod.get('k1', mod.get('k')),
                'wv1': g * mod.get('v1', mod.get('v')),
                'wk2': g * mod.get('k2', mod.get('k')),
                'wv2': g * mod.get('v2', mod.get('v'))}

    m1w = fold(p['m1'], g1)
    a1w = {'wq': g1 * p['a1']['q'] * SC, 'wk': g1 * p['a1']['k'],
           'wv': g1 * p['a1']['v']}
    m_attn1, a_attn1 = attn_launch(cx1, cx2, m1w, a1w)

    # L3: out-proj + residual + LN(n2)
    masl, aasl = tok_slices(m_attn1), tok_slices(a_attn1)
    r3 = _run(progs['mid'],
              [{'m_asl': masl[c], 'a_asl': aasl[c],
                'x1s': x1sl[c], 'x2s': x2sl[c],
                'wo1': _bf(p['m1']['ow']), 'wo2': _bf(p['a1']['ow'])}
               for c in range(NC)])
    x1psl = [r3[c]['x1n'] for c in range(NC)]
    x2psl = [r3[c]['x2n'] for c in range(NC)]
    cx1p = np.concatenate([r3[c]['cx1s'] for c in range(NC)], axis=1)
    cx2p = np.concatenate([r3[c]['cx2s'] for c in range(NC)], axis=1)

    m2w = fold(p['m2'], g2)
    a2w = {'wq': g2 * p['a2']['q'] * SC, 'wk': g2 * p['a2']['k'],
           'wv': g2 * p['a2']['v']}
    m_attn2, a_attn2 = attn_launch(cx1p, cx2p, m2w, a2w)

    # L5: out-proj + residual + LN(n3) + GEGLU FFs
    masl2, aasl2 = tok_slices(m_attn2), tok_slices(a_attn2)
    r5 = _run(progs['tail'],
              [{'m_asl': masl2[c], 'a_asl': aasl2[c],
                'x1s': x1psl[c], 'x2s': x2psl[c],
                'wo1': _bf(p['m2']['ow']), 'wo2': _bf(p['a2']['ow']),
                'w1m': _bf(g3 * p['mff']['w1']), 'w2m': _bf(p['mff']['w2']),
                'w1f': _bf(g3 * p['ff']['w1']), 'w2f': _bf(p['ff']['w2'])}
               for c in range(NC)])
    y1 = np.concatenate([r5[c]['y1s'].T for c in range(NC)], axis=0)[None]
    y2 = np.concatenate([r5[c]['y2s'].T for c in range(NC)], axis=0)[None]
    return (y1, y2)
